# revision 19
# baseline (speedup 1.0000x reference)
"""H2GCN forward pass on 8 Trainium2 NeuronCores (Bass/Tile SPMD kernel).

v3 strategy (binary-fp8 conv1 + resident B1 + AllGather stats):
  - Nodes sharded across 8 cores (1024 rows each). BOTH SpMM phases use the
    exact binary decomposition adj = dis_i * B * dis_j with B in fp8 (0/1
    exact): conv1 contracts dis_j-scaled h (fp16 lhsT ring) against fp8 B
    tiles, and the output columns are scaled by dis_i fused into the
    PSUM->SBUF copy. This removes the fp16 adjT tensors entirely (-33.6MB
    HBM per core).
  - B1^T [8192,1024] fp8 is loaded ONCE and stays resident in SBUF (64KB per
    partition); conv2' reuses it with zero DMA. B2^T is streamed twice
    (conv1 half1 + conv2'), rings on the sync queue.
  - The BN stats sync is an AllGather (+7 local vector adds) instead of the
    Mesh AllReduce, which measured 51us+11us trigger for 4KB; gathers of
    this size measure ~7us.
  - BN scale c is folded into the WEIGHTS (12 [128,64] scaled copies of
    wTf blocks) instead of scaling zT in place: z-fin and the p1/p2
    pre-projections read raw zT with c-scaled weights.
  - The rank-2 d-correction (s0+bias + s1*rsA + s2*rsA2) is applied by a
    tiny [3,128]x[3,64] matmul accumulated INTO the output-transpose PSUM
    group, replacing the [O,R] f32 broadcast tensors.
  - p-AllGather stays chunked (4); unpacks moved to the vector DMA queue so
    collective triggers on gpsimd run back-to-back; bt2/pg streams don't
    share a queue with anything that blocks.
"""

import numpy as np
import ml_dtypes

import concourse.bass as bass
import concourse.mybir as mybir
import concourse.tile as tile
from concourse import bacc
from concourse.bass_utils import run_bass_kernel_spmd
from concourse.masks import make_identity

P = 128
NCORES = 8
BN_EPS = 1e-5

F16 = mybir.dt.float16
F32 = mybir.dt.float32
F8 = mybir.dt.float8e4

FULL_CFG = dict(NT=8192, R=1024)
IN_CH = 512   # input features
H = 256       # hidden
H2 = 512      # 2*H (BN width)
O = 64        # output features
F = 7 * H     # 1792, JK concat width


def _nchunks(R):
    """Split the per-core node free-dim R into <=512 chunks (PSUM bank width)."""
    out = []
    s = 0
    while s < R:
        w = min(512, R - s)
        out.append((s, w))
        s += w
    return out


def build_program(NT, R):
    """Build the SPMD Bass program. NT = total nodes, R = rows per core."""
    KT = NT // P           # node k-tiles (contraction tiles)
    RT = R // P            # per-core node tiles
    NCH = _nchunks(R)
    NC2 = len(NCH)
    HM = H // P            # 2
    H2M = H2 // P          # 4
    FM = F // P            # 14
    INK = IN_CH // P       # 4
    NGC = 4                # p-AllGather chunks
    RTC = RT // NGC        # 2 node-tiles per gather chunk
    XW = NT // 16          # 512, x stream group width

    nc = bacc.Bacc("TRN2", target_bir_lowering=False, debug=False,
                   num_devices=NCORES)

    # --- I/O -------------------------------------------------------------
    XG = 16
    xTp = nc.dram_tensor("xTp", [P, NT * IN_CH // P], F16,
                         kind="ExternalInput")
    xT = nc.dram_tensor("xT", [IN_CH, R], F16, kind="ExternalInput")
    Bp1p = nc.dram_tensor("Bp1p", [NT // 2, 2 * R], F8, kind="ExternalInput")
    Bp2p = nc.dram_tensor("Bp2p", [NT // 2, 2 * R], F8, kind="ExternalInput")
    wTe = nc.dram_tensor("wTe", [IN_CH, H], F16, kind="ExternalInput")
    be = nc.dram_tensor("be", [P, HM], F32, kind="ExternalInput")
    bebc = nc.dram_tensor("bebc", [P, H], F32, kind="ExternalInput")
    wTf = nc.dram_tensor("wTf", [F, O], F16, kind="ExternalInput")
    bff = nc.dram_tensor("bff", [O, 1], F32, kind="ExternalInput")
    gam = nc.dram_tensor("gam", [P, H2M], F32, kind="ExternalInput")
    bet = nc.dram_tensor("bet", [P, H2M], F32, kind="ExternalInput")
    d1f = nc.dram_tensor("d1f", [P, KT], F32, kind="ExternalInput")
    d2f = nc.dram_tensor("d2f", [P, KT], F32, kind="ExternalInput")
    disP1 = nc.dram_tensor("disP1", [P, R], F16, kind="ExternalInput")
    disP2 = nc.dram_tensor("disP2", [P, R], F16, kind="ExternalInput")
    disr1 = nc.dram_tensor("disr1", [O, R], F16, kind="ExternalInput")
    disr2 = nc.dram_tensor("disr2", [O, R], F16, kind="ExternalInput")
    rk3 = nc.dram_tensor("rk3", [3, R], F16, kind="ExternalInput")
    dis1L = nc.dram_tensor("dis1L", [P, RT], F32, kind="ExternalInput")
    dis2L = nc.dram_tensor("dis2L", [P, RT], F32, kind="ExternalInput")
    out = nc.dram_tensor("out", [R, O], F32, kind="ExternalOutput")

    rg = [list(range(NCORES))]

    with tile.TileContext(nc) as tc:
        with (
            tc.tile_pool(name="const", bufs=1) as const,
            tc.tile_pool(name="feat", bufs=1) as feat,
            tc.tile_pool(name="tmp", bufs=2) as tmp,
            tc.tile_pool(name="stream", bufs=8) as stream,
            tc.tile_pool(name="ps", bufs=1, space="PSUM") as ps,
            tc.tile_pool(name="dram", bufs=1, space="DRAM") as dram,
        ):
            # --- embed-critical DMA first (sync queue) ------------------
            wTe_sb = const.tile([P, INK, H], F16, name="wTe_sb")
            nc.sync.dma_start(wTe_sb[:], wTe.ap().rearrange("(k p) m -> p k m", p=P))
            bebc_sb = const.tile([P, H], F32, name="bebc_sb")
            nc.sync.dma_start(bebc_sb[:], bebc.ap())

            # x streamed in 16 groups on the gpsimd DMA queue (fastest;
            # host-packed so each partition line is one 4KB contiguous run)
            xTp_t = xTp.ap().rearrange("p (g t w) -> p g t w", g=XG, t=INK)
            xts = []
            for g in range(XG):
                xt = stream.tile([P, INK, XW], F16, name=f"x_{g}", tag="xst",
                                 bufs=5)
                eng = nc.gpsimd if g % 2 == 0 else nc.scalar
                eng.dma_start(xt[:], xTp_t[:, g])
                xts.append(xt)

            # --- bulk constants (scalar queue, after the x groups) ------
            id16 = const.tile([P, P], F16, name="id16")
            make_identity(nc, id16)
            id32 = const.tile([P, P], F32, name="id32")
            make_identity(nc, id32)
            d1f_sb = const.tile([P, KT], F32, name="d1f_sb")
            nc.scalar.dma_start(d1f_sb[:], d1f.ap())
            d2f_sb = const.tile([P, KT], F32, name="d2f_sb")
            nc.scalar.dma_start(d2f_sb[:], d2f.ap())
            disP1_sb = const.tile([P, R], F16, name="disP1_sb")
            nc.scalar.dma_start(disP1_sb[:], disP1.ap())
            disP2_sb = const.tile([P, R], F16, name="disP2_sb")
            nc.scalar.dma_start(disP2_sb[:], disP2.ap())
            wTf_sb = const.tile([P, FM, O], F16, name="wTf_sb")
            nc.scalar.dma_start(wTf_sb[:], wTf.ap().rearrange("(k p) m -> p k m", p=P))
            bff_sb = const.tile([O, 1], F32, name="bff_sb")
            nc.scalar.dma_start(bff_sb[:], bff.ap())
            gam_sb = const.tile([P, H2M], F32, name="gam_sb")
            nc.scalar.dma_start(gam_sb[:], gam.ap())
            bet_sb = const.tile([P, H2M], F32, name="bet_sb")
            nc.scalar.dma_start(bet_sb[:], bet.ap())
            disr1_sb = const.tile([O, R], F16, name="disr1_sb")
            nc.scalar.dma_start(disr1_sb[:], disr1.ap())
            disr2_sb = const.tile([O, R], F16, name="disr2_sb")
            nc.scalar.dma_start(disr2_sb[:], disr2.ap())
            rk3_sb = const.tile([3, R], F16, name="rk3_sb")
            nc.scalar.dma_start(rk3_sb[:], rk3.ap())
            dis1L_sb = const.tile([P, RT], F32, name="dis1L_sb")
            nc.scalar.dma_start(dis1L_sb[:], dis1L.ap())
            dis2L_sb = const.tile([P, RT], F32, name="dis2L_sb")
            nc.scalar.dma_start(dis2L_sb[:], dis2L.ap())
            xT_sb = const.tile([P, INK, R], F16, name="xT_sb")
            nc.scalar.dma_start(xT_sb[:], xT.ap().rearrange("(k p) n -> p k n", p=P))
            be_sb = const.tile([P, HM], F32, name="be_sb")
            nc.scalar.dma_start(be_sb[:], be.ap())

            # B1^T resident fp8 (k-tile pairs, 2KB partition lines),
            # split across sync+scalar; x owns the gpsimd queue early
            Bsb1 = feat.tile([P, KT // 2, 2 * R], F8, name="Bsb1")
            Bp1p_t = Bp1p.ap().rearrange("(kk p) jr -> p kk jr", p=P)
            for c in range(8):
                nc.sync.dma_start(Bsb1[:, c * 4:(c + 1) * 4, :],
                                  Bp1p_t[:, c * 4:(c + 1) * 4, :])

            # --- phase B: replicated full embed, node-major -------------
            hfull_sb = feat.tile([P, KT, H], F16, name="hfull_sb")
            for k in range(KT):
                hps = ps.tile([P, H], F32, name=f"hps_{k}", tag=f"acc{k % 8}")
                g, off = k // (XW // P), (k % (XW // P)) * P
                for t in range(INK):
                    nc.tensor.matmul(
                        hps[:],
                        lhsT=xts[g][:, t, off:off + P],
                        rhs=wTe_sb[:, t, :],
                        start=(t == 0), stop=(t == INK - 1),
                    )
                nc.vector.tensor_tensor(
                    out=hfull_sb[:, k, :], in0=hps[:], in1=bebc_sb[:],
                    op=mybir.AluOpType.add)
                nc.vector.tensor_scalar_max(
                    hfull_sb[:, k, :], hfull_sb[:, k, :], 0.0)

            # --- phase D: conv1 via binary fp8 SpMM ---------------------
            # half0: A (B1 resident); half1: A2 (B2 streamed)
            zT_sb = feat.tile([P, H2M, R], F16, name="zT_sb")
            # per-half stats [sum_m0, sum_m1, sq_m0, sq_m1]; each half gets
            # its own AllGather so half0's hides under half1's compute
            stats = [tmp.tile([P, 2 * HM], F32, name=f"stat_{h}", bufs=1)
                     for h in range(2)]

            def conv1_half(half, dfsb, disPsb, bank0):
                stat_sb = stats[half]
                zps = {}
                for m in range(HM):
                    for ci in range(NC2):
                        zps[(m, ci)] = ps.tile(
                            [P, 512], F32, name=f"zps_{half}_{m}_{ci}",
                            tag=f"acc{bank0 + m * NC2 + ci}")
                for kk in range(KT // 2):
                    if half == 0:
                        rhs_tile = Bsb1[:, kk, :]
                    else:
                        bt = stream.tile([P, 2 * R], F8, name=f"b2_{kk}",
                                         tag="b2r", bufs=5)
                        nc.sync.dma_start(
                            bt[:], Bp2p[kk * P:(kk + 1) * P, :])
                        rhs_tile = bt[:]
                    for j in range(2):
                        k = 2 * kk + j
                        ht = tmp.tile([P, H], F16, name=f"ht_{half}_{k}",
                                      tag=f"hr{half}", bufs=4)
                        nc.vector.tensor_scalar_mul(
                            ht[:], hfull_sb[:, k, :], dfsb[:, k:k + 1])
                        for m in range(HM):
                            for ci, (cs, cw) in enumerate(NCH):
                                nc.tensor.matmul(
                                    zps[(m, ci)][:, :cw],
                                    lhsT=ht[:, m * P:(m + 1) * P],
                                    rhs=rhs_tile[:, j * R + cs:j * R + cs + cw],
                                    start=(k == 0), stop=(k == KT - 1),
                                )
                for m in range(HM):
                    f = half * HM + m
                    for ci, (cs, cw) in enumerate(NCH):
                        # fused dis_i column scaling in the PSUM->SBUF copy
                        nc.vector.tensor_mul(
                            out=zT_sb[:, f, cs:cs + cw],
                            in0=zps[(m, ci)][:, :cw],
                            in1=disPsb[:, cs:cs + cw])
                    nc.vector.tensor_reduce(
                        out=stat_sb[:, m:m + 1], in_=zT_sb[:, f, :],
                        axis=mybir.AxisListType.X, op=mybir.AluOpType.add)
                    sq2 = tmp.tile([P, R], F16, name="sq2", tag="sq", bufs=2)
                    nc.scalar.activation(
                        sq2[:], zT_sb[:, f, :],
                        mybir.ActivationFunctionType.Square,
                        accum_out=stat_sb[:, HM + m:HM + m + 1])
                # per-half stats AllGather (half0's hides under half1)
                ar_in = dram.tile([P, 2 * HM], F32, name=f"ar_in_{half}")
                nc.gpsimd.dma_start(ar_in[:], stat_sb[:])
                ar_out = dram.tile([NCORES, P, 2 * HM], F32,
                                   name=f"ar_out_{half}", addr_space="Shared")
                nc.gpsimd.collective_compute(
                    "AllGather", mybir.AluOpType.bypass, replica_groups=rg,
                    ins=[ar_in.opt()], outs=[ar_out.opt()],
                )
                st8 = tmp.tile([P, NCORES, 2 * HM], F32,
                               name=f"stat8_{half}", bufs=2, tag="st8")
                nc.gpsimd.dma_start(
                    st8[:], ar_out.rearrange("c p f -> p c f"))
                return st8

            st8A = conv1_half(0, d1f_sb, disP1_sb, 0)
            st8B = conv1_half(1, d2f_sb, disP2_sb, 4)
            # local 8-way reduction AFTER both halves (keeps the vector
            # queue free for half1's h-scales while gather-A completes)
            # 8-way reduction on gpsimd (idle engine) so the scheduler
            # cannot interleave these waits into the vector h-scale stream
            reds = []
            for half, st8 in ((0, st8A), (1, st8B)):
                red = tmp.tile([P, 2 * HM], F32, name=f"red_{half}",
                               bufs=2, tag="red")
                nc.gpsimd.tensor_tensor(
                    out=red[:], in0=st8[:, 0, :], in1=st8[:, 1, :],
                    op=mybir.AluOpType.add)
                for c in range(2, NCORES):
                    nc.gpsimd.tensor_tensor(
                        out=red[:], in0=red[:], in1=st8[:, c, :],
                        op=mybir.AluOpType.add)
                reds.append(red)
            redA, redB = reds

            # fill the gather window on tensor: local hT embed + fin h-block
            hT_sb = feat.tile([P, HM, R], F16, name="hT_sb")
            for m in range(HM):
                for ci, (cs, cw) in enumerate(NCH):
                    eps_t = ps.tile([P, 512], F32, name=f"eps_{m}_{ci}",
                                    tag=f"acc{(m * NC2 + ci) % 2}")
                    for t in range(INK):
                        nc.tensor.matmul(
                            eps_t[:, :cw],
                            lhsT=wTe_sb[:, t, m * P:(m + 1) * P],
                            rhs=xT_sb[:, t, cs:cs + cw],
                            start=(t == 0), stop=(t == INK - 1),
                        )
                    nc.scalar.activation(
                        hT_sb[:, m, cs:cs + cw], eps_t[:, :cw],
                        mybir.ActivationFunctionType.Relu,
                        bias=be_sb[:, m:m + 1],
                    )
            eps_sb = tmp.tile([P, 1], F32, name="eps_sb", bufs=1)
            nc.vector.memset(eps_sb[:], BN_EPS)
            warm = tmp.tile([P, 1], F32, name="warm", bufs=1)
            nc.scalar.activation(
                warm[:], eps_sb[:], mybir.ActivationFunctionType.Sqrt,
                bias=eps_sb[:])
            fin = {}
            for ci, (cs, cw) in enumerate(NCH):
                fin[ci] = ps.tile([O, 512], F32, name=f"fin_{ci}",
                                  tag=f"acc{6 + ci}")
                for t in range(HM):
                    nc.tensor.matmul(
                        fin[ci][:, :cw], lhsT=wTf_sb[:, t, :],
                        rhs=hT_sb[:, t, cs:cs + cw],
                        start=(t == 0), stop=False)

            # BN coefficients c, d (features 0:2 from half0, 2:4 from half1)
            cmean = tmp.tile([P, H2M], F32, name="cmean", bufs=1)
            nc.scalar.mul(cmean[:, 0:HM], redA[:, 0:HM], 1.0 / NT)
            nc.scalar.mul(cmean[:, HM:H2M], redB[:, 0:HM], 1.0 / NT)
            cvar = tmp.tile([P, H2M], F32, name="cvar", bufs=1)
            nc.scalar.mul(cvar[:, 0:HM], redA[:, HM:2 * HM], 1.0 / NT)
            nc.scalar.mul(cvar[:, HM:H2M], redB[:, HM:2 * HM], 1.0 / NT)
            msq = tmp.tile([P, H2M], F32, name="msq", bufs=1)
            nc.vector.tensor_mul(out=msq[:], in0=cmean[:], in1=cmean[:])
            nc.vector.tensor_tensor(
                out=cvar[:], in0=cvar[:], in1=msq[:],
                op=mybir.AluOpType.subtract)
            cstd = tmp.tile([P, H2M], F32, name="cstd", bufs=1)
            nc.scalar.activation(
                cstd[:], cvar[:], mybir.ActivationFunctionType.Sqrt,
                bias=eps_sb[:])
            crstd = tmp.tile([P, H2M], F32, name="crstd", bufs=1)
            nc.vector.reciprocal(crstd[:], cstd[:])
            c_t = tmp.tile([P, H2M], F32, name="c_t", bufs=1)
            nc.vector.tensor_mul(out=c_t[:], in0=crstd[:], in1=gam_sb[:])
            d_t = tmp.tile([P, H2M], F32, name="d_t", bufs=1)
            nc.vector.tensor_mul(out=d_t[:], in0=cmean[:], in1=c_t[:])
            nc.vector.tensor_tensor(
                out=d_t[:], in0=bet_sb[:], in1=d_t[:],
                op=mybir.AluOpType.subtract)
            d16 = tmp.tile([P, H2M], F16, name="d16", bufs=1)
            nc.vector.tensor_copy(out=d16[:], in_=d_t[:])

            # c folded into weights: 12 scaled blocks
            # [0:4] = z-fin blocks, [4:8] = p1 blocks, [8:12] = p2 blocks
            # p-blocks first: they gate the p-projection -> AllGather path
            wTfs = tmp.tile([P, 3 * H2M, O], F16, name="wTfs", bufs=1)
            for j, base in enumerate((HM + H2M, HM + 2 * H2M)):
                for t in range(H2M):
                    nc.vector.tensor_scalar_mul(
                        wTfs[:, (j + 1) * H2M + t, :], wTf_sb[:, base + t, :],
                        c_t[:, t:t + 1])

            # --- phase G: pre-projections p1, p2 (raw zT, scaled W) -----
            # per-ci: project -> transpose -> pack -> gather, so chunk 0's
            # AllGather launches after only half the projection work
            pT_sb = tmp.tile([P, R], F16, name="pT_sb", bufs=1)
            pcat_nm = tmp.tile([P, RT, P], F16, name="pcat_nm", bufs=1)
            pg_outs = []
            for ci, (cs, cw) in enumerate(NCH):
                for j in range(2):
                    pps = ps.tile([O, 512], F32, name=f"pps_{j}_{ci}",
                                  tag=f"acc{4 + j}")
                    for t in range(H2M):
                        nc.tensor.matmul(
                            pps[:, :cw],
                            lhsT=wTfs[:, (j + 1) * H2M + t, :],
                            rhs=zT_sb[:, t, cs:cs + cw],
                            start=(t == 0), stop=(t == H2M - 1))
                    nc.vector.tensor_copy(
                        out=pT_sb[j * O:(j + 1) * O, cs:cs + cw],
                        in_=pps[:, :cw])
                for nt in range(ci * RT // NC2, (ci + 1) * RT // NC2):
                    tps = ps.tile([P, P], F16, name=f"ptp_{nt}",
                                  tag=f"acc{2 + nt % 2}")
                    nc.tensor.transpose(
                        tps[:], pT_sb[:, nt * P:(nt + 1) * P], id16[:])
                    nc.vector.tensor_scalar_mul(
                        pcat_nm[:, nt, 0:O], tps[:, 0:O],
                        dis1L_sb[:, nt:nt + 1])
                    nc.vector.tensor_scalar_mul(
                        pcat_nm[:, nt, O:P], tps[:, O:P],
                        dis2L_sb[:, nt:nt + 1])
                for gc in range(ci * NGC // NC2, (ci + 1) * NGC // NC2):
                    pg_in = dram.tile([RTC * P, P], F16, name=f"pg_in_{gc}")
                    nc.scalar.dma_start(
                        pg_in.rearrange("(nt p) f -> p nt f", p=P),
                        pcat_nm[:, gc * RTC:(gc + 1) * RTC, :])
                    pg_o = dram.tile([NCORES, RTC * P, P], F16,
                                     name=f"pg_out_{gc}",
                                     addr_space="Shared")
                    nc.gpsimd.collective_compute(
                        "AllGather", mybir.AluOpType.bypass,
                        replica_groups=rg,
                        ins=[pg_in.opt()], outs=[pg_o.opt()],
                    )
                    pg_outs.append(pg_o)

            # --- filler work during the p-AllGathers --------------------
            # z-block c-scaled weights, s vectors, z-block final matmuls
            for t in range(H2M):
                nc.vector.tensor_scalar_mul(
                    wTfs[:, t, :], wTf_sb[:, HM + t, :], c_t[:, t:t + 1])
            # s vectors (rank-2 d-correction), srt = [s1; s2; s0+bias]
            s_cols = tmp.tile([O, 3], F32, name="s_cols", bufs=1)
            for j, base in enumerate((HM, HM + H2M, HM + 2 * H2M)):
                sps = ps.tile([O, 1], F32, name=f"sps_{j}", tag="acc2")
                for t in range(H2M):
                    nc.tensor.matmul(
                        sps[:], lhsT=wTf_sb[:, base + t, :],
                        rhs=d16[:, t:t + 1],
                        start=(t == 0), stop=(t == H2M - 1))
                nc.vector.tensor_copy(out=s_cols[:, j:j + 1], in_=sps[:])
            s16 = tmp.tile([O, 3], F16, name="s16", bufs=1)
            nc.vector.tensor_copy(out=s16[:, 0:1], in_=s_cols[:, 1:2])
            nc.vector.tensor_copy(out=s16[:, 1:2], in_=s_cols[:, 2:3])
            s0b = tmp.tile([O, 1], F32, name="s0b", bufs=1)
            nc.vector.tensor_add(out=s0b[:], in0=s_cols[:, 0:1], in1=bff_sb[:])
            nc.vector.tensor_copy(out=s16[:, 2:3], in_=s0b[:])
            srt_ps = ps.tile([3, O], F16, name="srt_ps", tag="acc3")
            nc.tensor.transpose(srt_ps[:], s16[:], id16[:O, :O])
            srt_sb = tmp.tile([3, O], F16, name="srt_sb", bufs=1)
            nc.vector.tensor_copy(out=srt_sb[:], in_=srt_ps[:])

            # z-block final matmuls (raw zT x c-scaled weights)
            for ci, (cs, cw) in enumerate(NCH):
                for t in range(H2M):
                    nc.tensor.matmul(
                        fin[ci][:, :cw], lhsT=wTfs[:, t, :],
                        rhs=zT_sb[:, t, cs:cs + cw],
                        start=False, stop=(t == H2M - 1))

            # unpacks on scalar (packs were issued early, so no blocking);
            # gathered p tiles live in a ring consumed in-order by conv2'
            pg_tiles = {}
            for gc in range(NGC):
                for cr in range(NCORES):
                    pgt = stream.tile([P, RTC, P], F16,
                                      name=f"pg_{gc}_{cr}", tag="pgr",
                                      bufs=12)
                    nc.scalar.dma_start(
                        pgt[:],
                        pg_outs[gc][cr].rearrange("(nt p) f -> p nt f", p=P))
                    pg_tiles[(gc, cr)] = pgt

            # --- phase H: conv2' (B1 resident, B2 streamed) -------------
            q1 = {}
            q2 = {}
            for ci in range(NC2):
                q1[ci] = ps.tile([O, 512], F32, name=f"q1_{ci}",
                                 tag=f"acc{4 + ci}")
                q2[ci] = ps.tile([O, 512], F32, name=f"q2_{ci}",
                                 tag=f"acc{ci}")
            for gc in range(NGC):
                for cr in range(NCORES):
                    kk = cr * RT // 2 + gc
                    bt2 = stream.tile([P, 2 * R], F8, name=f"c2b_{kk}",
                                      tag="b2c", bufs=5)
                    nc.sync.dma_start(bt2[:], Bp2p[kk * P:(kk + 1) * P, :])
                    pgt = pg_tiles[(gc, cr)]
                    for j in range(2):
                        k = 2 * kk + j
                        first = (gc == 0 and cr == 0 and j == 0)
                        last = (gc == NGC - 1 and cr == NCORES - 1
                                and j == 1)
                        for ci, (cs, cw) in enumerate(NCH):
                            nc.tensor.matmul(
                                q1[ci][:, :cw],
                                lhsT=pgt[:, j, 0:O],
                                rhs=Bsb1[:, kk, j * R + cs:j * R + cs + cw],
                                start=first, stop=last)
                        for ci, (cs, cw) in enumerate(NCH):
                            nc.tensor.matmul(
                                q2[ci][:, :cw],
                                lhsT=pgt[:, j, O:P],
                                rhs=bt2[:, j * R + cs:j * R + cs + cw],
                                start=first, stop=last)

            # combine: out = fin + dis1_i*q1 + dis2_i*q2, transposed to
            # node-major with the rank-2 correction accumulated in PSUM
            out_t = out.ap().rearrange("(nt p) o -> p nt o", p=P)
            o_nm = tmp.tile([P, RT, O], F32, name="o_nm", bufs=1)
            for ci, (cs, cw) in enumerate(NCH):
                outsb = tmp.tile([O, 512], F32, name=f"outsb_{ci}",
                                 tag="outsb", bufs=2)
                t1 = tmp.tile([O, 512], F32, name=f"t1_{ci}", tag="cmb",
                              bufs=2)
                nc.vector.tensor_mul(
                    out=t1[:, :cw], in0=q1[ci][:, :cw],
                    in1=disr1_sb[:, cs:cs + cw])
                nc.vector.tensor_add(
                    out=outsb[:, :cw], in0=fin[ci][:, :cw], in1=t1[:, :cw])
                t2 = tmp.tile([O, 512], F32, name=f"t2_{ci}", tag="cmb",
                              bufs=2)
                nc.vector.tensor_mul(
                    out=t2[:, :cw], in0=q2[ci][:, :cw],
                    in1=disr2_sb[:, cs:cs + cw])
                nc.vector.tensor_add(
                    out=outsb[:, :cw], in0=outsb[:, :cw], in1=t2[:, :cw])
                for nt in range(ci * RT // NC2, (ci + 1) * RT // NC2):
                    lo = nt * P - cs
                    tps32 = ps.tile([P, O], F32, name=f"otp_{nt}",
                                    tag=f"acc{2 + nt % 2}")
                    nc.tensor.matmul(
                        tps32[:], lhsT=outsb[:, lo:lo + P],
                        rhs=id32[:O, :O], is_transpose=True,
                        start=True, stop=False)
                    nc.tensor.matmul(
                        tps32[:], lhsT=rk3_sb[:, nt * P:(nt + 1) * P],
                        rhs=srt_sb[:],
                        start=False, stop=True, skip_group_check=True)
                    nc.any.tensor_copy(out=o_nm[:, nt, :], in_=tps32[:])
                nc.sync.dma_start(
                    out_t[:, ci * RT // NC2:(ci + 1) * RT // NC2, :],
                    o_nm[:, ci * RT // NC2:(ci + 1) * RT // NC2, :])

    nc.compile()
    return nc


_PROGRAM_CACHE = {}


def _get_program(NT, R):
    key = (NT, R)
    if key not in _PROGRAM_CACHE:
        _PROGRAM_CACHE[key] = build_program(NT, R)
    return _PROGRAM_CACHE[key]


def make_in_maps(inputs, NT, R):
    """Shard full inputs into per-core input maps (host-side, numpy)."""
    RT = R // P
    KT = NT // P
    x = np.asarray(inputs["x"], np.float32)
    adj = np.asarray(inputs["adj_t"], np.float32)
    adj2 = np.asarray(inputs["adj_t2"], np.float32)
    we = np.asarray(inputs["w_embed"], np.float32)
    be = np.asarray(inputs["b_embed"], np.float32)
    gam = np.asarray(inputs["bn_gamma"], np.float32)
    bet = np.asarray(inputs["bn_beta"], np.float32)
    wf = np.asarray(inputs["w_fin"], np.float32)
    bf = np.asarray(inputs["b_fin"], np.float32)

    H2M = H2 // P
    KT = NT // P
    INK = IN_CH // P
    XG = 16
    XW = NT // XG
    # x packed so each SBUF partition line is one 4KB contiguous run:
    # xTp[p, g, t, w] = x[g*XW+w, t*P+p]
    xTp_h = np.ascontiguousarray(
        x.T.astype(np.float16).reshape(INK, P, XG, XW)
        .transpose(1, 2, 0, 3).reshape(P, -1))
    wTe_h = np.ascontiguousarray(we.T).astype(np.float16)
    be_h = np.ascontiguousarray(be.reshape(H // P, P).T).astype(np.float32)
    bebc_h = np.ascontiguousarray(
        np.broadcast_to(be[None, :], (P, H))).astype(np.float32)
    wTf_h = np.ascontiguousarray(wf.T).astype(np.float16)
    bff_h = np.ascontiguousarray(bf[:, None]).astype(np.float32)
    gam_h = np.ascontiguousarray(gam.reshape(H2M, P).T).astype(np.float32)
    bet_h = np.ascontiguousarray(bet.reshape(H2M, P).T).astype(np.float32)

    # binary decomposition of the normalized adjacencies
    B1 = adj > 0
    B2 = adj2 > 0
    dg1 = B1.sum(1).astype(np.float32)
    dg2 = B2.sum(1).astype(np.float32)
    dis1 = np.where(dg1 > 0, 1.0 / np.sqrt(np.maximum(dg1, 1e-12)), 0.0
                    ).astype(np.float32)
    dis2 = np.where(dg2 > 0, 1.0 / np.sqrt(np.maximum(dg2, 1e-12)), 0.0
                    ).astype(np.float32)
    bdt = ml_dtypes.float8_e4m3
    d1f_h = np.ascontiguousarray(dis1.reshape(KT, P).T).astype(np.float32)
    d2f_h = np.ascontiguousarray(dis2.reshape(KT, P).T).astype(np.float32)

    in_maps = []
    for r in range(NCORES):
        rows = slice(r * R, (r + 1) * R)
        rk3_h = np.ascontiguousarray(np.stack([
            adj[rows].sum(1), adj2[rows].sum(1), np.ones(R, np.float32),
        ])).astype(np.float16)
        B1T = B1[rows].T.astype(bdt)
        B2T = B2[rows].T.astype(bdt)
        in_maps.append({
            "xTp": xTp_h,
            "xT": np.ascontiguousarray(x[rows].T).astype(np.float16),
            "Bp1p": np.ascontiguousarray(
                B1T.reshape(KT // 2, 2, P, R).transpose(0, 2, 1, 3)
                .reshape(NT // 2, 2 * R)),
            "Bp2p": np.ascontiguousarray(
                B2T.reshape(KT // 2, 2, P, R).transpose(0, 2, 1, 3)
                .reshape(NT // 2, 2 * R)),
            "wTe": wTe_h, "be": be_h, "bebc": bebc_h, "wTf": wTf_h,
            "bff": bff_h, "gam": gam_h, "bet": bet_h,
            "d1f": d1f_h, "d2f": d2f_h,
            "disP1": np.ascontiguousarray(
                np.broadcast_to(dis1[rows][None, :], (P, R))
            ).astype(np.float16),
            "disP2": np.ascontiguousarray(
                np.broadcast_to(dis2[rows][None, :], (P, R))
            ).astype(np.float16),
            "disr1": np.ascontiguousarray(
                np.broadcast_to(dis1[rows][None, :], (O, R))
            ).astype(np.float16),
            "disr2": np.ascontiguousarray(
                np.broadcast_to(dis2[rows][None, :], (O, R))
            ).astype(np.float16),
            "rk3": rk3_h,
            "dis1L": np.ascontiguousarray(
                dis1[rows].reshape(RT, P).T).astype(np.float32),
            "dis2L": np.ascontiguousarray(
                dis2[rows].reshape(RT, P).T).astype(np.float32),
        })
    return in_maps


def kernel(**inputs):
    NT, R = FULL_CFG["NT"], FULL_CFG["R"]
    nc = _get_program(NT, R)
    in_maps = make_in_maps(inputs, NT, R)
    res = run_bass_kernel_spmd(nc, in_maps, core_ids=list(range(NCORES)))
    out = np.concatenate(
        [res.results[r]["out"] for r in range(NCORES)], axis=0)
    return out.astype(np.float32)


# revision 20
# speedup vs baseline: 1.0954x; 1.0954x over previous
"""H2GCN forward pass on 8 Trainium2 NeuronCores (Bass/Tile SPMD kernel).

v3 strategy (binary-fp8 conv1 + resident B1 + AllGather stats):
  - Nodes sharded across 8 cores (1024 rows each). BOTH SpMM phases use the
    exact binary decomposition adj = dis_i * B * dis_j with B in fp8 (0/1
    exact): conv1 contracts dis_j-scaled h (fp16 lhsT ring) against fp8 B
    tiles, and the output columns are scaled by dis_i fused into the
    PSUM->SBUF copy. This removes the fp16 adjT tensors entirely (-33.6MB
    HBM per core).
  - B1^T [8192,1024] fp8 is loaded ONCE and stays resident in SBUF (64KB per
    partition); conv2' reuses it with zero DMA. B2^T is streamed twice
    (conv1 half1 + conv2'), rings on the sync queue.
  - The BN stats sync is an AllGather (+7 local vector adds) instead of the
    Mesh AllReduce, which measured 51us+11us trigger for 4KB; gathers of
    this size measure ~7us.
  - BN scale c is folded into the WEIGHTS (12 [128,64] scaled copies of
    wTf blocks) instead of scaling zT in place: z-fin and the p1/p2
    pre-projections read raw zT with c-scaled weights.
  - The rank-2 d-correction (s0+bias + s1*rsA + s2*rsA2) is applied by a
    tiny [3,128]x[3,64] matmul accumulated INTO the output-transpose PSUM
    group, replacing the [O,R] f32 broadcast tensors.
  - p-AllGather stays chunked (4); unpacks moved to the vector DMA queue so
    collective triggers on gpsimd run back-to-back; bt2/pg streams don't
    share a queue with anything that blocks.
"""

import numpy as np
import ml_dtypes

import concourse.bass as bass
import concourse.mybir as mybir
import concourse.tile as tile
from concourse import bacc
from concourse.bass_utils import run_bass_kernel_spmd
from concourse.masks import make_identity

P = 128
NCORES = 8
BN_EPS = 1e-5

F16 = mybir.dt.float16
F32 = mybir.dt.float32
F8 = mybir.dt.float8e4

FULL_CFG = dict(NT=8192, R=1024)
IN_CH = 512   # input features
H = 256       # hidden
H2 = 512      # 2*H (BN width)
O = 64        # output features
F = 7 * H     # 1792, JK concat width


def _nchunks(R):
    """Split the per-core node free-dim R into <=512 chunks (PSUM bank width)."""
    out = []
    s = 0
    while s < R:
        w = min(512, R - s)
        out.append((s, w))
        s += w
    return out


def build_program(NT, R):
    """Build the SPMD Bass program. NT = total nodes, R = rows per core."""
    KT = NT // P           # node k-tiles (contraction tiles)
    RT = R // P            # per-core node tiles
    NCH = _nchunks(R)
    NC2 = len(NCH)
    HM = H // P            # 2
    H2M = H2 // P          # 4
    FM = F // P            # 14
    INK = IN_CH // P       # 4
    NGC = 4                # p-AllGather chunks
    RTC = RT // NGC        # 2 node-tiles per gather chunk
    XW = NT // 16          # 512, x stream group width

    nc = bacc.Bacc("TRN2", target_bir_lowering=False, debug=False,
                   num_devices=NCORES)

    # --- I/O -------------------------------------------------------------
    XG = 16
    xTp = nc.dram_tensor("xTp", [P, NT * IN_CH // P], F16,
                         kind="ExternalInput")
    xT = nc.dram_tensor("xT", [IN_CH, R], F16, kind="ExternalInput")
    Bp1p = nc.dram_tensor("Bp1p", [NT // 2, 2 * R], F8, kind="ExternalInput")
    Bp2p = nc.dram_tensor("Bp2p", [NT // 2, 2 * R], F8, kind="ExternalInput")
    wTe = nc.dram_tensor("wTe", [IN_CH, H], F16, kind="ExternalInput")
    be = nc.dram_tensor("be", [P, HM], F32, kind="ExternalInput")
    bebc = nc.dram_tensor("bebc", [P, H], F32, kind="ExternalInput")
    wTf = nc.dram_tensor("wTf", [F, O], F16, kind="ExternalInput")
    bff = nc.dram_tensor("bff", [O, 1], F32, kind="ExternalInput")
    gam = nc.dram_tensor("gam", [P, H2M], F32, kind="ExternalInput")
    bet = nc.dram_tensor("bet", [P, H2M], F32, kind="ExternalInput")
    d1f = nc.dram_tensor("d1f", [P, KT], F32, kind="ExternalInput")
    d2f = nc.dram_tensor("d2f", [P, KT], F32, kind="ExternalInput")
    disP1 = nc.dram_tensor("disP1", [P, R], F16, kind="ExternalInput")
    disP2 = nc.dram_tensor("disP2", [P, R], F16, kind="ExternalInput")
    disr1 = nc.dram_tensor("disr1", [O, R], F16, kind="ExternalInput")
    disr2 = nc.dram_tensor("disr2", [O, R], F16, kind="ExternalInput")
    rk3 = nc.dram_tensor("rk3", [3, R], F16, kind="ExternalInput")
    dis1L = nc.dram_tensor("dis1L", [P, RT], F32, kind="ExternalInput")
    dis2L = nc.dram_tensor("dis2L", [P, RT], F32, kind="ExternalInput")
    out = nc.dram_tensor("out", [R, O], F32, kind="ExternalOutput")

    rg = [list(range(NCORES))]

    with tile.TileContext(nc) as tc:
        with (
            tc.tile_pool(name="const", bufs=1) as const,
            tc.tile_pool(name="feat", bufs=1) as feat,
            tc.tile_pool(name="tmp", bufs=2) as tmp,
            tc.tile_pool(name="stream", bufs=8) as stream,
            tc.tile_pool(name="ps", bufs=1, space="PSUM") as ps,
            tc.tile_pool(name="dram", bufs=1, space="DRAM") as dram,
        ):
            # --- embed-critical DMA first (sync queue) ------------------
            wTe_sb = const.tile([P, INK, H], F16, name="wTe_sb")
            nc.sync.dma_start(wTe_sb[:], wTe.ap().rearrange("(k p) m -> p k m", p=P))
            bebc_sb = const.tile([P, H], F32, name="bebc_sb")
            nc.sync.dma_start(bebc_sb[:], bebc.ap())

            # x streamed in 16 groups on the gpsimd DMA queue (fastest;
            # host-packed so each partition line is one 4KB contiguous run)
            xTp_t = xTp.ap().rearrange("p (g t w) -> p g t w", g=XG, t=INK)
            xts = []
            for g in range(XG):
                xt = stream.tile([P, INK, XW], F16, name=f"x_{g}", tag="xst",
                                 bufs=5)
                eng = nc.gpsimd if g % 2 == 0 else nc.sync
                eng.dma_start(xt[:], xTp_t[:, g])
                xts.append(xt)

            # --- bulk constants (scalar queue, after the x groups) ------
            id16 = const.tile([P, P], F16, name="id16")
            make_identity(nc, id16)
            id32 = const.tile([P, P], F32, name="id32")
            make_identity(nc, id32)
            d1f_sb = const.tile([P, KT], F32, name="d1f_sb")
            nc.scalar.dma_start(d1f_sb[:], d1f.ap())
            d2f_sb = const.tile([P, KT], F32, name="d2f_sb")
            nc.scalar.dma_start(d2f_sb[:], d2f.ap())
            disP1_sb = const.tile([P, R], F16, name="disP1_sb")
            nc.scalar.dma_start(disP1_sb[:], disP1.ap())
            disP2_sb = const.tile([P, R], F16, name="disP2_sb")
            nc.scalar.dma_start(disP2_sb[:], disP2.ap())
            wTf_sb = const.tile([P, FM, O], F16, name="wTf_sb")
            nc.scalar.dma_start(wTf_sb[:], wTf.ap().rearrange("(k p) m -> p k m", p=P))
            bff_sb = const.tile([O, 1], F32, name="bff_sb")
            nc.scalar.dma_start(bff_sb[:], bff.ap())
            gam_sb = const.tile([P, H2M], F32, name="gam_sb")
            nc.scalar.dma_start(gam_sb[:], gam.ap())
            bet_sb = const.tile([P, H2M], F32, name="bet_sb")
            nc.scalar.dma_start(bet_sb[:], bet.ap())
            disr1_sb = const.tile([O, R], F16, name="disr1_sb")
            nc.scalar.dma_start(disr1_sb[:], disr1.ap())
            disr2_sb = const.tile([O, R], F16, name="disr2_sb")
            nc.scalar.dma_start(disr2_sb[:], disr2.ap())
            rk3_sb = const.tile([3, R], F16, name="rk3_sb")
            nc.scalar.dma_start(rk3_sb[:], rk3.ap())
            dis1L_sb = const.tile([P, RT], F32, name="dis1L_sb")
            nc.scalar.dma_start(dis1L_sb[:], dis1L.ap())
            dis2L_sb = const.tile([P, RT], F32, name="dis2L_sb")
            nc.scalar.dma_start(dis2L_sb[:], dis2L.ap())
            xT_sb = const.tile([P, INK, R], F16, name="xT_sb")
            nc.scalar.dma_start(xT_sb[:], xT.ap().rearrange("(k p) n -> p k n", p=P))
            be_sb = const.tile([P, HM], F32, name="be_sb")
            nc.scalar.dma_start(be_sb[:], be.ap())

            # B1^T resident fp8 (k-tile pairs, 2KB partition lines),
            # split across sync+scalar; x owns the gpsimd queue early
            Bsb1 = feat.tile([P, KT // 2, 2 * R], F8, name="Bsb1")
            Bp1p_t = Bp1p.ap().rearrange("(kk p) jr -> p kk jr", p=P)
            for c in range(8):
                nc.sync.dma_start(Bsb1[:, c * 4:(c + 1) * 4, :],
                                  Bp1p_t[:, c * 4:(c + 1) * 4, :])

            # --- phase B: replicated full embed, node-major -------------
            hfull_sb = feat.tile([P, KT, H], F16, name="hfull_sb")
            for k in range(KT):
                hps = ps.tile([P, H], F32, name=f"hps_{k}", tag=f"acc{k % 8}")
                g, off = k // (XW // P), (k % (XW // P)) * P
                for t in range(INK):
                    nc.tensor.matmul(
                        hps[:],
                        lhsT=xts[g][:, t, off:off + P],
                        rhs=wTe_sb[:, t, :],
                        start=(t == 0), stop=(t == INK - 1),
                    )
                nc.vector.tensor_tensor(
                    out=hfull_sb[:, k, :], in0=hps[:], in1=bebc_sb[:],
                    op=mybir.AluOpType.add)
                nc.scalar.activation(
                    hfull_sb[:, k, :], hfull_sb[:, k, :],
                    mybir.ActivationFunctionType.Relu)

            # --- phase D: conv1 via binary fp8 SpMM ---------------------
            # half0: A (B1 resident); half1: A2 (B2 streamed)
            zT_sb = feat.tile([P, H2M, R], F16, name="zT_sb")
            # per-half stats [sum_m0, sum_m1, sq_m0, sq_m1]; each half gets
            # its own AllGather so half0's hides under half1's compute
            stats = [tmp.tile([P, 2 * HM], F32, name=f"stat_{h}", bufs=1)
                     for h in range(2)]

            def conv1_half(half, dfsb, disPsb, bank0):
                stat_sb = stats[half]
                zps = {}
                for m in range(HM):
                    for ci in range(NC2):
                        zps[(m, ci)] = ps.tile(
                            [P, 512], F32, name=f"zps_{half}_{m}_{ci}",
                            tag=f"acc{bank0 + m * NC2 + ci}")
                for kk in range(KT // 2):
                    if half == 0:
                        rhs_tile = Bsb1[:, kk, :]
                    else:
                        bt = stream.tile([P, 2 * R], F8, name=f"b2_{kk}",
                                         tag="b2r", bufs=5)
                        nc.sync.dma_start(
                            bt[:], Bp2p[kk * P:(kk + 1) * P, :])
                        rhs_tile = bt[:]
                    for j in range(2):
                        k = 2 * kk + j
                        ht = tmp.tile([P, H], F16, name=f"ht_{half}_{k}",
                                      tag=f"hr{half}", bufs=4)
                        nc.vector.tensor_scalar_mul(
                            ht[:], hfull_sb[:, k, :], dfsb[:, k:k + 1])
                        for m in range(HM):
                            for ci, (cs, cw) in enumerate(NCH):
                                nc.tensor.matmul(
                                    zps[(m, ci)][:, :cw],
                                    lhsT=ht[:, m * P:(m + 1) * P],
                                    rhs=rhs_tile[:, j * R + cs:j * R + cs + cw],
                                    start=(k == 0), stop=(k == KT - 1),
                                )
                for m in range(HM):
                    f = half * HM + m
                    for ci, (cs, cw) in enumerate(NCH):
                        # fused dis_i column scaling in the PSUM->SBUF copy
                        nc.vector.tensor_mul(
                            out=zT_sb[:, f, cs:cs + cw],
                            in0=zps[(m, ci)][:, :cw],
                            in1=disPsb[:, cs:cs + cw])
                    nc.vector.tensor_reduce(
                        out=stat_sb[:, m:m + 1], in_=zT_sb[:, f, :],
                        axis=mybir.AxisListType.X, op=mybir.AluOpType.add)
                    sq2 = tmp.tile([P, R], F16, name="sq2", tag="sq", bufs=2)
                    nc.scalar.activation(
                        sq2[:], zT_sb[:, f, :],
                        mybir.ActivationFunctionType.Square,
                        accum_out=stat_sb[:, HM + m:HM + m + 1])
                # per-half stats AllGather (half0's hides under half1)
                ar_in = dram.tile([P, 2 * HM], F32, name=f"ar_in_{half}")
                nc.gpsimd.dma_start(ar_in[:], stat_sb[:])
                ar_out = dram.tile([NCORES, P, 2 * HM], F32,
                                   name=f"ar_out_{half}", addr_space="Shared")
                nc.gpsimd.collective_compute(
                    "AllGather", mybir.AluOpType.bypass, replica_groups=rg,
                    ins=[ar_in.opt()], outs=[ar_out.opt()],
                )
                st8 = tmp.tile([P, NCORES, 2 * HM], F32,
                               name=f"stat8_{half}", bufs=2, tag="st8")
                nc.gpsimd.dma_start(
                    st8[:], ar_out.rearrange("c p f -> p c f"))
                return st8

            st8A = conv1_half(0, d1f_sb, disP1_sb, 0)
            st8B = conv1_half(1, d2f_sb, disP2_sb, 4)
            # local 8-way reduction AFTER both halves (keeps the vector
            # queue free for half1's h-scales while gather-A completes)
            # 8-way reduction on gpsimd (idle engine) so the scheduler
            # cannot interleave these waits into the vector h-scale stream
            reds = []
            for half, st8 in ((0, st8A), (1, st8B)):
                red = tmp.tile([P, 2 * HM], F32, name=f"red_{half}",
                               bufs=2, tag="red")
                nc.gpsimd.tensor_tensor(
                    out=red[:], in0=st8[:, 0, :], in1=st8[:, 1, :],
                    op=mybir.AluOpType.add)
                for c in range(2, NCORES):
                    nc.gpsimd.tensor_tensor(
                        out=red[:], in0=red[:], in1=st8[:, c, :],
                        op=mybir.AluOpType.add)
                reds.append(red)
            redA, redB = reds

            # fill the gather window on tensor: local hT embed + fin h-block
            hT_sb = feat.tile([P, HM, R], F16, name="hT_sb")
            for m in range(HM):
                for ci, (cs, cw) in enumerate(NCH):
                    eps_t = ps.tile([P, 512], F32, name=f"eps_{m}_{ci}",
                                    tag=f"acc{(m * NC2 + ci) % 2}")
                    for t in range(INK):
                        nc.tensor.matmul(
                            eps_t[:, :cw],
                            lhsT=wTe_sb[:, t, m * P:(m + 1) * P],
                            rhs=xT_sb[:, t, cs:cs + cw],
                            start=(t == 0), stop=(t == INK - 1),
                        )
                    nc.scalar.activation(
                        hT_sb[:, m, cs:cs + cw], eps_t[:, :cw],
                        mybir.ActivationFunctionType.Relu,
                        bias=be_sb[:, m:m + 1],
                    )
            eps_sb = tmp.tile([P, 1], F32, name="eps_sb", bufs=1)
            nc.vector.memset(eps_sb[:], BN_EPS)
            warm = tmp.tile([P, 1], F32, name="warm", bufs=1)
            nc.scalar.activation(
                warm[:], eps_sb[:], mybir.ActivationFunctionType.Sqrt,
                bias=eps_sb[:])
            fin = {}
            for ci, (cs, cw) in enumerate(NCH):
                fin[ci] = ps.tile([O, 512], F32, name=f"fin_{ci}",
                                  tag=f"acc{6 + ci}")
                for t in range(HM):
                    nc.tensor.matmul(
                        fin[ci][:, :cw], lhsT=wTf_sb[:, t, :],
                        rhs=hT_sb[:, t, cs:cs + cw],
                        start=(t == 0), stop=False)

            # BN coefficients c, d (features 0:2 from half0, 2:4 from half1)
            cmean = tmp.tile([P, H2M], F32, name="cmean", bufs=1)
            nc.scalar.mul(cmean[:, 0:HM], redA[:, 0:HM], 1.0 / NT)
            nc.scalar.mul(cmean[:, HM:H2M], redB[:, 0:HM], 1.0 / NT)
            cvar = tmp.tile([P, H2M], F32, name="cvar", bufs=1)
            nc.scalar.mul(cvar[:, 0:HM], redA[:, HM:2 * HM], 1.0 / NT)
            nc.scalar.mul(cvar[:, HM:H2M], redB[:, HM:2 * HM], 1.0 / NT)
            msq = tmp.tile([P, H2M], F32, name="msq", bufs=1)
            nc.vector.tensor_mul(out=msq[:], in0=cmean[:], in1=cmean[:])
            nc.vector.tensor_tensor(
                out=cvar[:], in0=cvar[:], in1=msq[:],
                op=mybir.AluOpType.subtract)
            cstd = tmp.tile([P, H2M], F32, name="cstd", bufs=1)
            nc.scalar.activation(
                cstd[:], cvar[:], mybir.ActivationFunctionType.Sqrt,
                bias=eps_sb[:])
            crstd = tmp.tile([P, H2M], F32, name="crstd", bufs=1)
            nc.vector.reciprocal(crstd[:], cstd[:])
            c_t = tmp.tile([P, H2M], F32, name="c_t", bufs=1)
            nc.vector.tensor_mul(out=c_t[:], in0=crstd[:], in1=gam_sb[:])
            d_t = tmp.tile([P, H2M], F32, name="d_t", bufs=1)
            nc.vector.tensor_mul(out=d_t[:], in0=cmean[:], in1=c_t[:])
            nc.vector.tensor_tensor(
                out=d_t[:], in0=bet_sb[:], in1=d_t[:],
                op=mybir.AluOpType.subtract)
            d16 = tmp.tile([P, H2M], F16, name="d16", bufs=1)
            nc.vector.tensor_copy(out=d16[:], in_=d_t[:])

            # c folded into weights: 12 scaled blocks
            # [0:4] = z-fin blocks, [4:8] = p1 blocks, [8:12] = p2 blocks
            # p-blocks first: they gate the p-projection -> AllGather path
            wTfs = tmp.tile([P, 3 * H2M, O], F16, name="wTfs", bufs=1)
            for j, base in enumerate((HM + H2M, HM + 2 * H2M)):
                for t in range(H2M):
                    nc.vector.tensor_scalar_mul(
                        wTfs[:, (j + 1) * H2M + t, :], wTf_sb[:, base + t, :],
                        c_t[:, t:t + 1])

            # --- phase G: pre-projections p1, p2 (raw zT, scaled W) -----
            # per-ci: project -> transpose -> pack -> gather, so chunk 0's
            # AllGather launches after only half the projection work
            pT_sb = tmp.tile([P, R], F16, name="pT_sb", bufs=1)
            pcat_nm = tmp.tile([P, RT, P], F16, name="pcat_nm", bufs=1)
            pg_outs = []
            for ci, (cs, cw) in enumerate(NCH):
                for j in range(2):
                    pps = ps.tile([O, 512], F32, name=f"pps_{j}_{ci}",
                                  tag=f"acc{4 + j}")
                    for t in range(H2M):
                        nc.tensor.matmul(
                            pps[:, :cw],
                            lhsT=wTfs[:, (j + 1) * H2M + t, :],
                            rhs=zT_sb[:, t, cs:cs + cw],
                            start=(t == 0), stop=(t == H2M - 1))
                    nc.vector.tensor_copy(
                        out=pT_sb[j * O:(j + 1) * O, cs:cs + cw],
                        in_=pps[:, :cw])
                for nt in range(ci * RT // NC2, (ci + 1) * RT // NC2):
                    tps = ps.tile([P, P], F16, name=f"ptp_{nt}",
                                  tag=f"acc{2 + nt % 2}")
                    nc.tensor.transpose(
                        tps[:], pT_sb[:, nt * P:(nt + 1) * P], id16[:])
                    nc.vector.tensor_scalar_mul(
                        pcat_nm[:, nt, 0:O], tps[:, 0:O],
                        dis1L_sb[:, nt:nt + 1])
                    nc.vector.tensor_scalar_mul(
                        pcat_nm[:, nt, O:P], tps[:, O:P],
                        dis2L_sb[:, nt:nt + 1])
                for gc in range(ci * NGC // NC2, (ci + 1) * NGC // NC2):
                    pg_in = dram.tile([RTC * P, P], F16, name=f"pg_in_{gc}")
                    nc.scalar.dma_start(
                        pg_in.rearrange("(nt p) f -> p nt f", p=P),
                        pcat_nm[:, gc * RTC:(gc + 1) * RTC, :])
                    pg_o = dram.tile([NCORES, RTC * P, P], F16,
                                     name=f"pg_out_{gc}",
                                     addr_space="Shared")
                    nc.gpsimd.collective_compute(
                        "AllGather", mybir.AluOpType.bypass,
                        replica_groups=rg,
                        ins=[pg_in.opt()], outs=[pg_o.opt()],
                    )
                    pg_outs.append(pg_o)

            # --- filler work during the p-AllGathers --------------------
            # z-block c-scaled weights, s vectors, z-block final matmuls
            for t in range(H2M):
                nc.vector.tensor_scalar_mul(
                    wTfs[:, t, :], wTf_sb[:, HM + t, :], c_t[:, t:t + 1])
            # s vectors (rank-2 d-correction), srt = [s1; s2; s0+bias]
            s_cols = tmp.tile([O, 3], F32, name="s_cols", bufs=1)
            for j, base in enumerate((HM, HM + H2M, HM + 2 * H2M)):
                sps = ps.tile([O, 1], F32, name=f"sps_{j}", tag="acc2")
                for t in range(H2M):
                    nc.tensor.matmul(
                        sps[:], lhsT=wTf_sb[:, base + t, :],
                        rhs=d16[:, t:t + 1],
                        start=(t == 0), stop=(t == H2M - 1))
                nc.vector.tensor_copy(out=s_cols[:, j:j + 1], in_=sps[:])
            s16 = tmp.tile([O, 3], F16, name="s16", bufs=1)
            nc.vector.tensor_copy(out=s16[:, 0:1], in_=s_cols[:, 1:2])
            nc.vector.tensor_copy(out=s16[:, 1:2], in_=s_cols[:, 2:3])
            s0b = tmp.tile([O, 1], F32, name="s0b", bufs=1)
            nc.vector.tensor_add(out=s0b[:], in0=s_cols[:, 0:1], in1=bff_sb[:])
            nc.vector.tensor_copy(out=s16[:, 2:3], in_=s0b[:])
            srt_ps = ps.tile([3, O], F16, name="srt_ps", tag="acc3")
            nc.tensor.transpose(srt_ps[:], s16[:], id16[:O, :O])
            srt_sb = tmp.tile([3, O], F16, name="srt_sb", bufs=1)
            nc.vector.tensor_copy(out=srt_sb[:], in_=srt_ps[:])

            # z-block final matmuls (raw zT x c-scaled weights)
            for ci, (cs, cw) in enumerate(NCH):
                for t in range(H2M):
                    nc.tensor.matmul(
                        fin[ci][:, :cw], lhsT=wTfs[:, t, :],
                        rhs=zT_sb[:, t, cs:cs + cw],
                        start=False, stop=(t == H2M - 1))

            # unpacks on scalar (packs were issued early, so no blocking);
            # gathered p tiles live in a ring consumed in-order by conv2'
            pg_tiles = {}
            for gc in range(NGC):
                for cr in range(NCORES):
                    pgt = stream.tile([P, RTC, P], F16,
                                      name=f"pg_{gc}_{cr}", tag="pgr",
                                      bufs=12)
                    nc.scalar.dma_start(
                        pgt[:],
                        pg_outs[gc][cr].rearrange("(nt p) f -> p nt f", p=P))
                    pg_tiles[(gc, cr)] = pgt

            # --- phase H: conv2' (B1 resident, B2 streamed) -------------
            q1 = {}
            q2 = {}
            for ci in range(NC2):
                q1[ci] = ps.tile([O, 512], F32, name=f"q1_{ci}",
                                 tag=f"acc{4 + ci}")
                q2[ci] = ps.tile([O, 512], F32, name=f"q2_{ci}",
                                 tag=f"acc{ci}")
            for gc in range(NGC):
                for cr in range(NCORES):
                    kk = cr * RT // 2 + gc
                    bt2 = stream.tile([P, 2 * R], F8, name=f"c2b_{kk}",
                                      tag="b2c", bufs=5)
                    nc.sync.dma_start(bt2[:], Bp2p[kk * P:(kk + 1) * P, :])
                    pgt = pg_tiles[(gc, cr)]
                    for j in range(2):
                        k = 2 * kk + j
                        first = (gc == 0 and cr == 0 and j == 0)
                        last = (gc == NGC - 1 and cr == NCORES - 1
                                and j == 1)
                        for ci, (cs, cw) in enumerate(NCH):
                            nc.tensor.matmul(
                                q1[ci][:, :cw],
                                lhsT=pgt[:, j, 0:O],
                                rhs=Bsb1[:, kk, j * R + cs:j * R + cs + cw],
                                start=first, stop=last)
                        for ci, (cs, cw) in enumerate(NCH):
                            nc.tensor.matmul(
                                q2[ci][:, :cw],
                                lhsT=pgt[:, j, O:P],
                                rhs=bt2[:, j * R + cs:j * R + cs + cw],
                                start=first, stop=last)

            # combine: out = fin + dis1_i*q1 + dis2_i*q2, transposed to
            # node-major with the rank-2 correction accumulated in PSUM
            out_t = out.ap().rearrange("(nt p) o -> p nt o", p=P)
            o_nm = tmp.tile([P, RT, O], F32, name="o_nm", bufs=1)
            for ci, (cs, cw) in enumerate(NCH):
                outsb = tmp.tile([O, 512], F32, name=f"outsb_{ci}",
                                 tag="outsb", bufs=2)
                t1 = tmp.tile([O, 512], F32, name=f"t1_{ci}", tag="cmb",
                              bufs=2)
                nc.vector.tensor_mul(
                    out=t1[:, :cw], in0=q1[ci][:, :cw],
                    in1=disr1_sb[:, cs:cs + cw])
                nc.vector.tensor_add(
                    out=outsb[:, :cw], in0=fin[ci][:, :cw], in1=t1[:, :cw])
                t2 = tmp.tile([O, 512], F32, name=f"t2_{ci}", tag="cmb",
                              bufs=2)
                nc.vector.tensor_mul(
                    out=t2[:, :cw], in0=q2[ci][:, :cw],
                    in1=disr2_sb[:, cs:cs + cw])
                nc.vector.tensor_add(
                    out=outsb[:, :cw], in0=outsb[:, :cw], in1=t2[:, :cw])
                for nt in range(ci * RT // NC2, (ci + 1) * RT // NC2):
                    lo = nt * P - cs
                    tps32 = ps.tile([P, O], F32, name=f"otp_{nt}",
                                    tag=f"acc{2 + nt % 2}")
                    nc.tensor.matmul(
                        tps32[:], lhsT=outsb[:, lo:lo + P],
                        rhs=id32[:O, :O], is_transpose=True,
                        start=True, stop=False)
                    nc.tensor.matmul(
                        tps32[:], lhsT=rk3_sb[:, nt * P:(nt + 1) * P],
                        rhs=srt_sb[:],
                        start=False, stop=True, skip_group_check=True)
                    nc.any.tensor_copy(out=o_nm[:, nt, :], in_=tps32[:])
                nc.sync.dma_start(
                    out_t[:, ci * RT // NC2:(ci + 1) * RT // NC2, :],
                    o_nm[:, ci * RT // NC2:(ci + 1) * RT // NC2, :])

    nc.compile()
    return nc


_PROGRAM_CACHE = {}


def _get_program(NT, R):
    key = (NT, R)
    if key not in _PROGRAM_CACHE:
        _PROGRAM_CACHE[key] = build_program(NT, R)
    return _PROGRAM_CACHE[key]


def make_in_maps(inputs, NT, R):
    """Shard full inputs into per-core input maps (host-side, numpy)."""
    RT = R // P
    KT = NT // P
    x = np.asarray(inputs["x"], np.float32)
    adj = np.asarray(inputs["adj_t"], np.float32)
    adj2 = np.asarray(inputs["adj_t2"], np.float32)
    we = np.asarray(inputs["w_embed"], np.float32)
    be = np.asarray(inputs["b_embed"], np.float32)
    gam = np.asarray(inputs["bn_gamma"], np.float32)
    bet = np.asarray(inputs["bn_beta"], np.float32)
    wf = np.asarray(inputs["w_fin"], np.float32)
    bf = np.asarray(inputs["b_fin"], np.float32)

    H2M = H2 // P
    KT = NT // P
    INK = IN_CH // P
    XG = 16
    XW = NT // XG
    # x packed so each SBUF partition line is one 4KB contiguous run:
    # xTp[p, g, t, w] = x[g*XW+w, t*P+p]
    xTp_h = np.ascontiguousarray(
        x.T.astype(np.float16).reshape(INK, P, XG, XW)
        .transpose(1, 2, 0, 3).reshape(P, -1))
    wTe_h = np.ascontiguousarray(we.T).astype(np.float16)
    be_h = np.ascontiguousarray(be.reshape(H // P, P).T).astype(np.float32)
    bebc_h = np.ascontiguousarray(
        np.broadcast_to(be[None, :], (P, H))).astype(np.float32)
    wTf_h = np.ascontiguousarray(wf.T).astype(np.float16)
    bff_h = np.ascontiguousarray(bf[:, None]).astype(np.float32)
    gam_h = np.ascontiguousarray(gam.reshape(H2M, P).T).astype(np.float32)
    bet_h = np.ascontiguousarray(bet.reshape(H2M, P).T).astype(np.float32)

    # binary decomposition of the normalized adjacencies
    B1 = adj > 0
    B2 = adj2 > 0
    dg1 = B1.sum(1).astype(np.float32)
    dg2 = B2.sum(1).astype(np.float32)
    dis1 = np.where(dg1 > 0, 1.0 / np.sqrt(np.maximum(dg1, 1e-12)), 0.0
                    ).astype(np.float32)
    dis2 = np.where(dg2 > 0, 1.0 / np.sqrt(np.maximum(dg2, 1e-12)), 0.0
                    ).astype(np.float32)
    bdt = ml_dtypes.float8_e4m3
    d1f_h = np.ascontiguousarray(dis1.reshape(KT, P).T).astype(np.float32)
    d2f_h = np.ascontiguousarray(dis2.reshape(KT, P).T).astype(np.float32)

    in_maps = []
    for r in range(NCORES):
        rows = slice(r * R, (r + 1) * R)
        rk3_h = np.ascontiguousarray(np.stack([
            adj[rows].sum(1), adj2[rows].sum(1), np.ones(R, np.float32),
        ])).astype(np.float16)
        B1T = B1[rows].T.astype(bdt)
        B2T = B2[rows].T.astype(bdt)
        in_maps.append({
            "xTp": xTp_h,
            "xT": np.ascontiguousarray(x[rows].T).astype(np.float16),
            "Bp1p": np.ascontiguousarray(
                B1T.reshape(KT // 2, 2, P, R).transpose(0, 2, 1, 3)
                .reshape(NT // 2, 2 * R)),
            "Bp2p": np.ascontiguousarray(
                B2T.reshape(KT // 2, 2, P, R).transpose(0, 2, 1, 3)
                .reshape(NT // 2, 2 * R)),
            "wTe": wTe_h, "be": be_h, "bebc": bebc_h, "wTf": wTf_h,
            "bff": bff_h, "gam": gam_h, "bet": bet_h,
            "d1f": d1f_h, "d2f": d2f_h,
            "disP1": np.ascontiguousarray(
                np.broadcast_to(dis1[rows][None, :], (P, R))
            ).astype(np.float16),
            "disP2": np.ascontiguousarray(
                np.broadcast_to(dis2[rows][None, :], (P, R))
            ).astype(np.float16),
            "disr1": np.ascontiguousarray(
                np.broadcast_to(dis1[rows][None, :], (O, R))
            ).astype(np.float16),
            "disr2": np.ascontiguousarray(
                np.broadcast_to(dis2[rows][None, :], (O, R))
            ).astype(np.float16),
            "rk3": rk3_h,
            "dis1L": np.ascontiguousarray(
                dis1[rows].reshape(RT, P).T).astype(np.float32),
            "dis2L": np.ascontiguousarray(
                dis2[rows].reshape(RT, P).T).astype(np.float32),
        })
    return in_maps


def kernel(**inputs):
    NT, R = FULL_CFG["NT"], FULL_CFG["R"]
    nc = _get_program(NT, R)
    in_maps = make_in_maps(inputs, NT, R)
    res = run_bass_kernel_spmd(nc, in_maps, core_ids=list(range(NCORES)))
    out = np.concatenate(
        [res.results[r]["out"] for r in range(NCORES)], axis=0)
    return out.astype(np.float32)
